# revision 4
# baseline (speedup 1.0000x reference)
"""Trainium2 Bass kernel for ChannelwiseSlidingWindowDropout2D.

Reference semantics (see problem):
    bits  = (noise < 0.1)                      # [C, 58, 58]
    drop  = maxpool7x7(bits, pad=(6,6))        # [C, 64, 64]
    out   = x * (1 - drop)[None]               # [B, C, H, W], mask batch-shared

Equivalent formulation used here (exact, elementwise fp32):
    keep[c,y,x] = 1.0  iff  min over the 7x7 noise window covering (y,x) >= 0.1
    out = x * keep

Sharding: channels split across the 8 cores (32 channels per core). Each
core receives x[:, c0:c0+32] flattened to [1024, 4096] and its 32-channel
noise slice [32, 58, 58]. On-chip, the noise is replicated onto all 128
SBUF partitions (partition p handles channel p % 32) so the separable
min-pool and the final multiply run at full partition width.

Min-pool is separable and decomposed with window doubling (1->2->4->7):
    T1 = min(P, shift1(P)); T2 = min(T1, shift2(T1)); W7 = min(T2, shift3(T2))
applied along W then along H on a 1.0-padded [70, 70] plane per channel.
"""

import numpy as np

B, C, H, W = 32, 256, 64, 64
WIN = 7
DROP_PROB = 0.1
HV, WV = H - WIN + 1, W - WIN + 1  # 58, 58
N_CORES = 8
C_PER_CORE = C // N_CORES  # 32
ROWS = B * C_PER_CORE      # 1024 rows of [64*64] per core
PAD = H + WIN - 1          # 70: zero/one-padded plane side

_CACHE = {}


def _build():
    import concourse.bass as bass
    import concourse.tile as tile
    from concourse import bacc, mybir

    f32 = mybir.dt.float32
    op_min = mybir.AluOpType.min
    op_mul = mybir.AluOpType.mult
    op_ge = mybir.AluOpType.is_ge

    nc = bacc.Bacc("TRN2", target_bir_lowering=False, debug=False)

    x_d = nc.declare_dram_parameter("xs", [ROWS, H, W], f32, isOutput=False)
    n_d = nc.declare_dram_parameter("ns", [C_PER_CORE, HV, WV], f32, isOutput=False)
    y_d = nc.declare_dram_parameter("y", [ROWS, H, W], f32, isOutput=True)

    with tile.TileContext(nc) as tc:
        with (
            tc.tile_pool(name="mpool", bufs=1) as mpool,
            tc.tile_pool(name="tpool", bufs=1) as tpool,
            tc.tile_pool(name="xpool", bufs=6) as xpool,
        ):
            # ---- mask: separable 7x7 min-pool over 1.0-padded noise ----
            P = tpool.tile([128, PAD, PAD], f32, tag="A")
            nc.vector.memset(P[:], 1.0)
            # replicate the 32-channel noise slice onto 4 partition groups with
            # a single stride-0 broadcast DMA (keeps P's writer count low: one
            # HW instruction only supports a couple of sync waits downstream)
            nc.sync.dma_start(
                out=P[:, WIN - 1 : WIN - 1 + HV, WIN - 1 : WIN - 1 + WV],
                in_=n_d[:, :, :].unsqueeze(0).broadcast_to([4, C_PER_CORE, HV, WV]),
            )

            # W-direction min over window 7 (rows 6..64 only; pad rows are 1.0)
            T1 = tpool.tile([128, HV, PAD - 1], f32, tag="B")  # [58, 69]
            nc.vector.tensor_tensor(
                out=T1[:], in0=P[:, 6:64, 0:69], in1=P[:, 6:64, 1:70], op=op_min
            )
            T2 = tpool.tile([128, HV, PAD - 3], f32, tag="A")  # [58, 67]
            nc.vector.tensor_tensor(
                out=T2[:], in0=T1[:, :, 0:67], in1=T1[:, :, 2:69], op=op_min
            )
            W7 = tpool.tile([128, PAD, W], f32, tag="B")  # [70, 64]
            nc.vector.memset(W7[:, 0:6, :], 1.0)
            nc.vector.memset(W7[:, 64:70, :], 1.0)
            nc.vector.tensor_tensor(
                out=W7[:, 6:64, :], in0=T2[:, :, 0:64], in1=T2[:, :, 3:67], op=op_min
            )

            # H-direction min over window 7
            U1 = tpool.tile([128, PAD - 1, W], f32, tag="A")  # [69, 64]
            nc.vector.tensor_tensor(
                out=U1[:], in0=W7[:, 0:69, :], in1=W7[:, 1:70, :], op=op_min
            )
            U2 = tpool.tile([128, PAD - 3, W], f32, tag="B")  # [67, 64]
            nc.vector.tensor_tensor(
                out=U2[:], in0=U1[:, 0:67, :], in1=U1[:, 2:69, :], op=op_min
            )
            Mm = tpool.tile([128, H, W], f32, tag="A")  # [64, 64]
            nc.vector.tensor_tensor(
                out=Mm[:], in0=U2[:, 0:64, :], in1=U2[:, 3:67, :], op=op_min
            )

            # keep-mask = (min >= DROP_PROB) -> 1.0 / 0.0
            M = mpool.tile([128, H, W], f32)
            nc.vector.tensor_scalar(
                out=M[:], in0=Mm[:], scalar1=DROP_PROB, scalar2=None, op0=op_ge
            )

            # ---- stream x tiles: load, multiply by mask, store ----
            for t in range(ROWS // 128):
                xt = xpool.tile([128, H, W], f32, tag="xt")
                nc.sync.dma_start(out=xt[:], in_=x_d[128 * t : 128 * (t + 1)])
                nc.vector.tensor_tensor(out=xt[:], in0=xt[:], in1=M[:], op=op_mul)
                nc.scalar.dma_start(out=y_d[128 * t : 128 * (t + 1)], in_=xt[:])

    nc.compile()
    return nc


def _get_nc():
    if "nc" not in _CACHE:
        _CACHE["nc"] = _build()
    return _CACHE["nc"]


def kernel(x: np.ndarray, noise: np.ndarray) -> np.ndarray:
    from concourse.bass_utils import run_bass_kernel_spmd

    x = np.asarray(x, dtype=np.float32)
    noise = np.asarray(noise, dtype=np.float32)

    nc = _get_nc()
    in_maps = []
    for i in range(N_CORES):
        c0 = i * C_PER_CORE
        xs = np.ascontiguousarray(x[:, c0 : c0 + C_PER_CORE]).reshape(ROWS, H, W)
        ns = np.ascontiguousarray(noise[c0 : c0 + C_PER_CORE])
        in_maps.append({"xs": xs, "ns": ns})

    trace = bool(_CACHE.get("trace", False))
    res = run_bass_kernel_spmd(
        nc, in_maps, core_ids=list(range(N_CORES)), trace=trace,
        **_CACHE.get("spmd_kwargs", {}),
    )
    _CACHE["last_results"] = res

    out = np.empty((B, C, H, W), dtype=np.float32)
    for i in range(N_CORES):
        c0 = i * C_PER_CORE
        out[:, c0 : c0 + C_PER_CORE] = res.results[i]["y"].reshape(
            B, C_PER_CORE, H, W
        )
    return out


# revision 5
# speedup vs baseline: 1.3075x; 1.3075x over previous
"""Trainium2 Bass kernel for ChannelwiseSlidingWindowDropout2D.

Reference semantics (see problem):
    bits  = (noise < 0.1)                      # [C, 58, 58]
    drop  = maxpool7x7(bits, pad=(6,6))        # [C, 64, 64]
    out   = x * (1 - drop)[None]               # [B, C, H, W], mask batch-shared

Equivalent formulation used here (exact, elementwise fp32):
    keep[c,y,x] = 1.0  iff  min over the 7x7 noise window covering (y,x) >= 0.1
    out = x * keep

Sharding: channels split across the 8 cores (32 channels per core). Each
core receives x[:, c0:c0+32] flattened to [1024, 64, 64] plus its noise
slice padded with 1.0 to the [70, 70] window-sliding plane and replicated
onto all 128 SBUF partitions host-side (partition p handles channel
p % 32), so the on-chip mask pipeline is one contiguous DMA followed by a
separable min-pool at full partition width.

Min-pool is separable and decomposed with window doubling (1->2->4->7):
    T1 = min(P, shift1(P)); T2 = min(T1, shift2(T1)); W7 = min(T2, shift3(T2))
applied along W then along H on the 1.0-padded [70, 70] plane per channel.
"""

import numpy as np

B, C, H, W = 32, 256, 64, 64
WIN = 7
DROP_PROB = 0.1
HV, WV = H - WIN + 1, W - WIN + 1  # 58, 58
N_CORES = 8
C_PER_CORE = C // N_CORES  # 32
ROWS = B * C_PER_CORE      # 1024 rows of [64, 64] per core
PAD = H + WIN - 1          # 70: 1.0-padded plane side
N_TILES = ROWS // 128      # 8 streaming tiles per core
GPSIMD_TILES = 2           # tail tiles multiplied on GpSimd instead of DVE

_CACHE = {}


def _build():
    import concourse.tile as tile
    from concourse import bacc, mybir

    f32 = mybir.dt.float32
    op_min = mybir.AluOpType.min
    op_mul = mybir.AluOpType.mult
    op_ge = mybir.AluOpType.is_ge

    nc = bacc.Bacc("TRN2", target_bir_lowering=False, debug=False)

    x_d = nc.declare_dram_parameter("xs", [ROWS, H, W], f32, isOutput=False)
    n_d = nc.declare_dram_parameter("np", [128, PAD * PAD], f32, isOutput=False)
    y_d = nc.declare_dram_parameter("y", [ROWS, H, W], f32, isOutput=True)

    with tile.TileContext(nc) as tc:
        with (
            tc.tile_pool(name="mpool", bufs=1) as mpool,
            tc.tile_pool(name="tpool", bufs=1) as tpool,
            tc.tile_pool(name="xpool", bufs=N_TILES) as xpool,
        ):
            # ---- mask: separable 7x7 min-pool over the padded noise plane ----
            P = tpool.tile([128, PAD, PAD], f32, tag="A")
            nc.sync.dma_start(out=P[:], in_=n_d[:])

            # W-direction min over window 7 (doubling: 1 -> 2 -> 4 -> 7)
            T1 = tpool.tile([128, PAD, PAD - 1], f32, tag="B")  # [70, 69]
            nc.vector.tensor_tensor(
                out=T1[:], in0=P[:, :, 0:69], in1=P[:, :, 1:70], op=op_min
            )
            T2 = tpool.tile([128, PAD, PAD - 3], f32, tag="A")  # [70, 67]
            nc.vector.tensor_tensor(
                out=T2[:], in0=T1[:, :, 0:67], in1=T1[:, :, 2:69], op=op_min
            )
            W7 = tpool.tile([128, PAD, W], f32, tag="B")  # [70, 64]
            nc.vector.tensor_tensor(
                out=W7[:], in0=T2[:, :, 0:64], in1=T2[:, :, 3:67], op=op_min
            )

            # H-direction min over window 7
            U1 = tpool.tile([128, PAD - 1, W], f32, tag="A")  # [69, 64]
            nc.vector.tensor_tensor(
                out=U1[:], in0=W7[:, 0:69, :], in1=W7[:, 1:70, :], op=op_min
            )
            U2 = tpool.tile([128, PAD - 3, W], f32, tag="B")  # [67, 64]
            nc.vector.tensor_tensor(
                out=U2[:], in0=U1[:, 0:67, :], in1=U1[:, 2:69, :], op=op_min
            )
            Mm = tpool.tile([128, H, W], f32, tag="A")  # [64, 64]
            nc.vector.tensor_tensor(
                out=Mm[:], in0=U2[:, 0:64, :], in1=U2[:, 3:67, :], op=op_min
            )

            # keep-mask = (min >= DROP_PROB) -> 1.0 / 0.0
            M = mpool.tile([128, H, W], f32)
            nc.vector.tensor_scalar(
                out=M[:], in0=Mm[:], scalar1=DROP_PROB, scalar2=None, op0=op_ge
            )

            # ---- stream x tiles: load, multiply by mask, store ----
            for t in range(N_TILES):
                xt = xpool.tile([128, H, W], f32, tag="xt")
                nc.sync.dma_start(out=xt[:], in_=x_d[128 * t : 128 * (t + 1)])
                eng = nc.gpsimd if t >= N_TILES - GPSIMD_TILES else nc.vector
                eng.tensor_tensor(out=xt[:], in0=xt[:], in1=M[:], op=op_mul)
                nc.scalar.dma_start(out=y_d[128 * t : 128 * (t + 1)], in_=xt[:])

    nc.compile()
    return nc


def _get_nc():
    if "nc" not in _CACHE:
        _CACHE["nc"] = _build()
    return _CACHE["nc"]


def _pad_noise(noise_slice: np.ndarray) -> np.ndarray:
    """[32, 58, 58] -> [128, 70*70]: pad with 1.0 to [70, 70] (interior at
    [6:64, 6:64]) and replicate onto the 4 partition groups."""
    p = np.ones((C_PER_CORE, PAD, PAD), dtype=np.float32)
    p[:, WIN - 1 : WIN - 1 + HV, WIN - 1 : WIN - 1 + WV] = noise_slice
    p = p.reshape(C_PER_CORE, PAD * PAD)
    return np.tile(p, (4, 1))


def kernel(x: np.ndarray, noise: np.ndarray) -> np.ndarray:
    from concourse.bass_utils import run_bass_kernel_spmd

    x = np.asarray(x, dtype=np.float32)
    noise = np.asarray(noise, dtype=np.float32)

    nc = _get_nc()
    in_maps = []
    for i in range(N_CORES):
        c0 = i * C_PER_CORE
        xs = np.ascontiguousarray(x[:, c0 : c0 + C_PER_CORE]).reshape(ROWS, H, W)
        ns = _pad_noise(noise[c0 : c0 + C_PER_CORE])
        in_maps.append({"xs": xs, "np": ns})

    res = run_bass_kernel_spmd(nc, in_maps, core_ids=list(range(N_CORES)))
    _CACHE["last_results"] = res

    out = np.empty((B, C, H, W), dtype=np.float32)
    for i in range(N_CORES):
        c0 = i * C_PER_CORE
        out[:, c0 : c0 + C_PER_CORE] = res.results[i]["y"].reshape(
            B, C_PER_CORE, H, W
        )
    return out
